# revision 12
# baseline (speedup 1.0000x reference)
"""DLRM (multi-table EmbeddingBag + MLPs) on 8 Trainium2 NeuronCores.

Strategy: data-parallel over batch (512 bags/core); embedding tables
replicated in each core's HBM as 104 window tensors (26 tables x 4 windows
of 25000 rows, so local row ids fit dma_gather's int16 index limit).

One dma_gather per (table, window) covering all 512 bags, slots sorted by
bag, queues cycled 0..3 in strict program order (the tile scheduler locks
each DMASW sem lane to one queue; lane=i%8 / queue=i%4 is consistent).
Gathered fp32 rows are cast to bf16 on the Scalar engine.
Pooling: per 128-slot group, one bf16 matmul with the cast rows as the
stationary operand and a HOST-precomputed narrow one-hot bf16 "band"
matrix (DMA'd from DRAM) as the moving operand, accumulating feature-major
into one PSUM bank per table ([64 feats, 512 bags], per-element has_written
handles overlapping band ranges).  The Vector engine is kept idle during
gathers (SWDGE descriptor generation and DVE 2-port ops hold an exclusive
lock on a shared SBUF port pair); PSUM drains, casts and MLP activations
run on the Scalar engine.  The top-MLP first layer is interleaved into the
table loop (each z-chunk's matmuls fire as soon as its two tables drain).
"""
import numpy as np

import concourse.bacc as bacc
import concourse.bass as bass
import concourse.mybir as mybir
import concourse.tile as tile
from concourse.bass_utils import run_bass_kernel_spmd

T = 26          # tables
R = 100000      # rows per table
E = 64          # embedding dim
B = 4096        # batch
L = 32          # lookups per bag
BOT = [256, 512, 256, 64]
TOP = [E * (1 + T), 512, 256, 1]   # 1728 -> 512 -> 256 -> 1
N_CORES = 8
B_CORE = B // N_CORES               # 512 bags per core
W_ROWS = 25000                      # window rows (<= int16 range)
N_WIN = R // W_ROWS                 # 4 windows per table
N_SEG = T * N_WIN
P = 128
ZF = TOP[0] + 64                    # 1792: zero-padded feature dim
NZCH = ZF // P                      # 14 z chunks


def _pack_idx_block(idx_i16):
    """[n] int16 (n % 16 == 0) -> [128, n//16]: j -> (j%16, j//16), x8."""
    n = idx_i16.size
    w = idx_i16.reshape(n // 16, 16).T
    return np.tile(w, (8, 1))


def _chunk_weights(wt):
    """W.T [din, dout] -> [128, (din/128)*dout] SBUF chunk layout."""
    din, dout = wt.shape
    nk = din // P
    return np.ascontiguousarray(
        wt.reshape(nk, P, dout).transpose(1, 0, 2).reshape(P, nk * dout))


def _chunk_bias(b):
    """[dout] -> [128, ceil(dout/128)]."""
    dout = b.size
    nch = -(-dout // P)
    buf = np.zeros(nch * P, np.float32)
    buf[:dout] = b
    return np.ascontiguousarray(buf.reshape(nch, P).T)


def _host_prep(x_indices):
    """Per-(table,window) segment packing, shared across cores.

    Returns:
      meta: list of 104 segments (cap, bases, widths) in (t, w) order;
            bases/widths are per-128-slot-group one-hot band column ranges
            (shared across cores; even base, even width).
      idx_mats: per-core [128, tot16] int16 packed gather indices (0 pad,
            discarded via zero band rows).
      band_mats: per-core [128, sbtot] bf16 one-hot bands.
      tot16, sbtot
    """
    bf16 = mybir.dt.np(mybir.dt.bfloat16)
    idx = np.asarray(x_indices).astype(np.int64)  # [T, B, L]
    meta = []
    per_seg_core = []
    for t in range(T):
        for w in range(N_WIN):
            percore = []
            for c in range(N_CORES):
                sub = idx[t, c * B_CORE:(c + 1) * B_CORE, :]   # [512, L]
                win = sub // W_ROWS
                bags, ls = np.nonzero(win == w)                # bag-sorted
                li = (sub[bags, ls] - w * W_ROWS).astype(np.int16)
                percore.append((li, bags.astype(np.int32)))
            n_max = max(li.size for li, _ in percore)
            cap = max(128, -(-n_max // 128) * 128)
            gn = cap // 128
            bases, widths = [], []
            for g in range(gn):
                lo, hi = B_CORE, -1
                for li, bags in percore:
                    seg = bags[g * 128:(g + 1) * 128]
                    if seg.size:
                        lo = min(lo, int(seg[0]))
                        hi = max(hi, int(seg[-1]))
                if hi < 0:
                    bases.append(0)
                    widths.append(2)
                    continue
                base = lo & ~1
                wd = hi - base + 1
                wd += wd & 1
                wd = min(wd, B_CORE - base)
                bases.append(base)
                widths.append(wd)
            meta.append((cap, bases, widths))
            per_seg_core.append(percore)

    tot16 = sum(cap // 16 for cap, _, _ in meta)
    sbtot = sum(sum(ws) for _, _, ws in meta)
    idx_mats = [np.zeros((P, tot16), np.int16) for _ in range(N_CORES)]
    band_mats = [np.zeros((P, sbtot), bf16) for _ in range(N_CORES)]
    o16 = 0
    ob = 0
    for s, (cap, bases, widths) in enumerate(meta):
        goff = np.concatenate([[0], np.cumsum(widths)[:-1]]) + ob
        basev = np.asarray(bases, np.int64)
        for c in range(N_CORES):
            li, bags = per_seg_core[s][c]
            buf = np.zeros(cap, np.int16)
            buf[:li.size] = li
            idx_mats[c][:, o16:o16 + cap // 16] = _pack_idx_block(buf)
            slots = np.arange(li.size)
            grp = slots // 128
            prt = slots % 128
            col = goff[grp] + (bags.astype(np.int64) - basev[grp])
            band_mats[c][prt, col] = 1.0
        o16 += cap // 16
        ob += sum(widths)
    return meta, idx_mats, band_mats, tot16, sbtot


def _build(meta, tot16, sbtot):
    nc = bacc.Bacc("TRN2", target_bir_lowering=False, debug=False,
                   enable_asserts=False, num_devices=N_CORES,
                   num_swdge_queues=4)
    dt = mybir.dt.float32
    bf = mybir.dt.bfloat16
    AF = mybir.ActivationFunctionType

    win_d = [nc.dram_tensor(f"win{t}_{w}", [W_ROWS, E], dt,
                            kind="ExternalInput").ap()
             for t in range(T) for w in range(N_WIN)]
    idx_d = nc.dram_tensor("idxs", [P, tot16], mybir.dt.int16,
                           kind="ExternalInput").ap()
    band_d = nc.dram_tensor("bands", [P, sbtot], bf,
                            kind="ExternalInput").ap()
    xt_d = nc.dram_tensor("xt", [BOT[0], B_CORE], dt,
                          kind="ExternalInput").ap()
    wdims = [(BOT[0], BOT[1]), (BOT[1], BOT[2]), (BOT[2], BOT[3]),
             (ZF, TOP[1]), (TOP[1], TOP[2]), (TOP[2], TOP[3])]
    w_d = [nc.dram_tensor(f"w{i}", [P, (din // P) * dout], dt,
                          kind="ExternalInput").ap()
           for i, (din, dout) in enumerate(wdims)]
    b_d = [nc.dram_tensor(f"b{i}", [P, -(-dout // P)], dt,
                          kind="ExternalInput").ap()
           for i, (_, dout) in enumerate(wdims)]
    out_d = nc.dram_tensor("y", [1, B_CORE], dt, kind="ExternalOutput").ap()

    max_gn = max(cap // 128 for cap, _, _ in meta)
    max_c16 = max(cap // 16 for cap, _, _ in meta)
    max_sbw = max(sum(ws) for _, _, ws in meta)

    with tile.TileContext(nc) as tc:
        with tc.tile_pool(name="const", bufs=1) as cpool, \
             tc.tile_pool(name="zp", bufs=1) as zp, \
             tc.tile_pool(name="ip", bufs=7) as ip, \
             tc.tile_pool(name="sp", bufs=6) as sp, \
             tc.tile_pool(name="gp", bufs=6) as gp, \
             tc.tile_pool(name="gbp", bufs=6) as gbp, \
             tc.tile_pool(name="w3p", bufs=3) as w3p, \
             tc.tile_pool(name="act", bufs=1) as actp, \
             tc.tile_pool(name="pps", bufs=4, space="PSUM") as pps, \
             tc.tile_pool(name="mps", bufs=4, space="PSUM") as mps:

            seg_tiles = {}

            def issue_seg(s):
                cap, bases, widths = meta[s]
                o16 = sum(m[0] // 16 for m in meta[:s])
                ob = sum(sum(m[2]) for m in meta[:s])
                c16 = cap // 16
                gn = cap // 128
                sbw = sum(widths)
                idx_t = ip.tile([P, c16], mybir.dt.int16, tag="idx",
                                name=f"idx{s}")
                nc.sync.dma_start(out=idx_t[:], in_=idx_d[:, o16:o16 + c16])
                sel_t = sp.tile([P, sbw], bf, tag="sel", name=f"sel{s}")
                nc.sync.dma_start(out=sel_t[:], in_=band_d[:, ob:ob + sbw])
                dst = gp.tile([P, gn, E], dt, tag="dst", name=f"dst{s}")
                nc.gpsimd.dma_gather(
                    out_ap=dst[:], in_ap=win_d[s][:],
                    idxs_ap=idx_t[:], num_idxs=cap, num_idxs_reg=cap,
                    elem_size=E, single_packet=False,
                    queue_num=s % 4)
                dstb = gbp.tile([P, gn, E], bf, tag="dstb", name=f"dstb{s}")
                nc.scalar.activation(out=dstb[:], in_=dst[:], func=AF.Copy)
                seg_tiles[s] = (dstb, sel_t, bases, widths)

            # head: first table's gathers go out before anything else
            issue_seg(0)
            issue_seg(1)

            def load(name, ap_dram, shape):
                t_ = cpool.tile(shape, dt, tag=name, name=name)
                nc.sync.dma_start(out=t_[:], in_=ap_dram)
                return t_

            xt = [load(f"xt{k}", xt_d[k * P:(k + 1) * P, :], [P, B_CORE])
                  for k in range(BOT[0] // P)]
            wts = {i: load(f"w{i}", w_d[i][:, :], [P, (din // P) * dout])
                   for i, (din, dout) in enumerate(wdims) if i != 3}
            bts = [load(f"b{i}", b_d[i][:, :], [P, -(-dout // P)])
                   for i, (_, dout) in enumerate(wdims)]

            # z chunks: one tile per 128-feature block of the top-MLP input
            zch = [zp.tile([P, B_CORE], dt, tag=f"z{k}", name=f"zch{k}")
                   for k in range(NZCH)]
            nc.vector.memset(zch[NZCH - 1][64:128, :], 0.0)

            def mlp_layer(src_aps, li, func, out_tag):
                din, dout = wdims[li]
                nk = din // P
                outs = []
                for m in range(-(-dout // P)):
                    mm = min(P, dout - m * P)
                    ps = mps.tile([P, B_CORE], dt, space="PSUM", tag="mlp",
                                  name=f"ps{out_tag}{m}")
                    for k in range(nk):
                        nc.tensor.matmul(
                            out=ps[:mm, :],
                            lhsT=wts[li][:, k * dout + m * P:
                                         k * dout + m * P + mm],
                            rhs=src_aps[k],
                            start=(k == 0), stop=(k == nk - 1))
                    o = actp.tile([P, B_CORE], dt, tag=f"{out_tag}{m}",
                                  name=f"a{out_tag}{m}")
                    nc.scalar.activation(out=o[:mm, :], in_=ps[:mm, :],
                                         func=func,
                                         bias=bts[li][0:mm, m:m + 1])
                    outs.append(o)
                return outs

            # bottom MLP (feature-major h.T tiles [128, 512])
            h1 = mlp_layer([t_[:, :] for t_ in xt], 0, AF.Relu, "h1")
            h2 = mlp_layer([t_[:, :] for t_ in h1], 1, AF.Relu, "h2")
            h3 = mlp_layer([t_[:, :] for t_ in h2], 2, AF.Relu, "h3")
            nc.scalar.activation(out=zch[0][0:64, :], in_=h3[0][0:64, :],
                                 func=AF.Copy)

            # top-MLP layer 0 state (interleaved into the table loop)
            dout0 = TOP[1]
            psm = [mps.tile([P, B_CORE], dt, space="PSUM", tag="mlp",
                            name=f"psm{m}") for m in range(dout0 // P)]

            def top0_chunk(k):
                w3c = w3p.tile([P, dout0], dt, tag="w3c", name=f"w3c{k}")
                nc.sync.dma_start(out=w3c[:],
                                  in_=w_d[3][:, k * dout0:(k + 1) * dout0])
                for m in range(dout0 // P):
                    nc.tensor.matmul(
                        out=psm[m][:, :],
                        lhsT=w3c[:, m * P:(m + 1) * P],
                        rhs=zch[k][:, :],
                        start=(k == 0), stop=(k == NZCH - 1))

            # embedding gather + band-matmul pooling
            for t in range(T):
                fbase = 64 + 64 * t
                ch = fbase // P
                prow = fbase % P
                ps = pps.tile([P, B_CORE], dt, space="PSUM", tag="pool",
                              name=f"pool{t}")
                for w in range(N_WIN):
                    s = t * N_WIN + w
                    if s not in seg_tiles:
                        issue_seg(s)
                n_mm = sum(len(meta[t * N_WIN + w][2]) for w in range(N_WIN))
                mm_i = 0
                for w in range(N_WIN):
                    s = t * N_WIN + w
                    dstb, sel_t, bases, widths = seg_tiles.pop(s)
                    off = 0
                    for g, (bs, wd) in enumerate(zip(bases, widths)):
                        nc.tensor.matmul(
                            out=ps[prow:prow + 64, bs:bs + wd],
                            lhsT=dstb[:, g, :],
                            rhs=sel_t[:, off:off + wd],
                            start=(mm_i == 0), stop=(mm_i == n_mm - 1))
                        mm_i += 1
                        off += wd
                nc.scalar.activation(out=zch[ch][prow:prow + 64, :],
                                     in_=ps[prow:prow + 64, :], func=AF.Copy)
                if t == 0:
                    top0_chunk(0)
                elif t % 2 == 0:
                    top0_chunk(t // 2)
                elif t == T - 1:
                    top0_chunk(NZCH - 1)

            # top MLP tail
            y1 = []
            for m in range(dout0 // P):
                o = actp.tile([P, B_CORE], dt, tag=f"y1{m}", name=f"y1t{m}")
                nc.scalar.activation(out=o[:, :], in_=psm[m][:, :],
                                     func=AF.Relu, bias=bts[3][0:P, m:m + 1])
                y1.append(o)
            y2 = mlp_layer([t_[:, :] for t_ in y1], 4, AF.Relu, "y2")
            ps = mps.tile([P, B_CORE], dt, space="PSUM", tag="mlp",
                          name="psfin")
            nk = TOP[2] // P
            for k in range(nk):
                nc.tensor.matmul(out=ps[:1, :],
                                 lhsT=wts[5][:, k * TOP[3]:k * TOP[3] + 1],
                                 rhs=y2[k][:, :],
                                 start=(k == 0), stop=(k == nk - 1))
            yo = actp.tile([1, B_CORE], dt, tag="yo", name="yo")
            nc.scalar.activation(out=yo[:], in_=ps[:1, :], func=AF.Sigmoid,
                                 bias=bts[5][0:1, 0:1])
            nc.sync.dma_start(out=out_d[:], in_=yo[:])

    nc.compile()
    return nc


def prepare(inputs):
    """Host prep + build + per-core input maps."""
    x_dense = np.asarray(inputs["x_dense"], np.float32)
    x_indices = np.asarray(inputs["x_indices"])
    emb = np.ascontiguousarray(np.asarray(inputs["emb_tables"], np.float32))

    meta, idx_mats, band_mats, tot16, sbtot = _host_prep(x_indices)
    nc = _build(meta, tot16, sbtot)

    common = {}
    for t in range(T):
        for w in range(N_WIN):
            common[f"win{t}_{w}"] = np.ascontiguousarray(
                emb[t, w * W_ROWS:(w + 1) * W_ROWS, :])
    w0 = np.asarray(inputs["top_w0"], np.float32)          # [512, 1728]
    w0p = np.zeros((TOP[1], ZF), np.float32)
    w0p[:, :TOP[0]] = w0
    wlist = [np.asarray(inputs["bot_w0"], np.float32).T,
             np.asarray(inputs["bot_w1"], np.float32).T,
             np.asarray(inputs["bot_w2"], np.float32).T,
             w0p.T,
             np.asarray(inputs["top_w1"], np.float32).T,
             np.asarray(inputs["top_w2"], np.float32).T]
    blist = [np.asarray(inputs["bot_b0"], np.float32),
             np.asarray(inputs["bot_b1"], np.float32),
             np.asarray(inputs["bot_b2"], np.float32),
             np.asarray(inputs["top_b0"], np.float32),
             np.asarray(inputs["top_b1"], np.float32),
             np.asarray(inputs["top_b2"], np.float32)]
    for i in range(6):
        common[f"w{i}"] = _chunk_weights(wlist[i])
        common[f"b{i}"] = _chunk_bias(blist[i])

    in_maps = []
    for c in range(N_CORES):
        m = dict(common)
        m["idxs"] = idx_mats[c]
        m["bands"] = band_mats[c]
        m["xt"] = np.ascontiguousarray(
            x_dense[c * B_CORE:(c + 1) * B_CORE, :].T)
        in_maps.append(m)
    return nc, in_maps


def kernel(**inputs):
    nc, in_maps = prepare(inputs)
    res = run_bass_kernel_spmd(nc, in_maps, core_ids=list(range(N_CORES)))
    y = np.empty((B, 1), np.float32)
    for c in range(N_CORES):
        y[c * B_CORE:(c + 1) * B_CORE, 0] = res.results[c]["y"][0]
    return y
